# revision 38
# baseline (speedup 1.0000x reference)
"""Trainium2 Bass kernel for the DifferentiableDAG forward pass.

reference math (D=128 vars, H=16 hidden, B=8192 batch):
    A[v,i]   = gumbel-sigmoid((2*logits[v,i] + g1 - g0)/t), zero diagonal
    hidden   = relu(einsum('biv,ivh', X[:,None,:]*A.T[None], W1) + b1)
    recon    = einsum('bih,ih', hidden, W2) + b2
    returns (recon, A)

Key identity: hidden_pre[b, i, h] = sum_v X[b,v] * (A[v,i]*W1[i,v,h]),
so the device builds W1eff[v, (i,h)] = A[v,i]*W1t[v, (i,h)] once and runs
a dense [B,128] @ [128,2048] matmul -- the [B,D,D] masked tensor from the
reference is never materialized (that's the memory-regime headroom).

Sharding: data-parallel on batch across 8 cores (1024 rows each);
adjacency/weights replicated, A and W1eff computed redundantly per core
(tiny). No collectives.

On-chip layout ("layout A"): (i_local*16+h) on PSUM partitions, batch on
the free axis, so b1/b2 are per-partition biases for the ACT engine.
  mm1:  psum1[128=(il,h), 512=b] = W1eff_chunk.T @ X.T    (16 chunks)
  relu: hid_c = relu(psum1 + b1_chunk)                    (ACT/DVE split)
  mm2:  psum2[128=i, 512=b] += W2blockdiag_chunk.T @ hid  (col-tiled 4x)
  out:  reconT = psum2 + b2, stored TRANSPOSED; host un-transposes.

Perf notes (measured on this container's trn2 + walrus):
- all matmul operands bf16 (fp32 runs two-pass at half rate); biases and
  PSUM accumulation stay fp32; recon error ~4e-3 scale-relative.
- X ships pre-transposed/interleaved and recon is written transposed
  (host un-transposes at unshard): no on-device transposes at all, and
  every DMA moves >=2KB contiguous runs per partition (small row-sized
  runs crawl at <5GB/s per queue from descriptor overhead).
- W1t ships with its (v==i) diagonal pre-zeroed, which takes the A-mask
  off the critical path; the masked A is produced separately for a_out.
- w1t streams in 4 quarters so mm1 chunk c only waits on quarter c//4;
  the A-chain inputs land first, packed and cast to bf16 (halves the
  gating DMA; A output error ~3e-3, still far under the gate).
- 40 dummy transposes of a zeroed scratch tile keep the PE busy across
  the whole DMA-bound ramp, so the HAM clock gate is at 2.4 GHz (and
  stays there) when the real matmul stream starts.
- mm2 uses tile_position column groups: chunk c owns vars
  off(c)=32*(c%4)+8*(c//4)..+8, so its 8 outputs land in col group c%4
  with a [128,32] stationary -- cheap loads, one accumulating psum.
- _split_waits(): this walrus encodes at most ONE sync-wait command per
  instruction, so extra waits are hoisted into standalone EventSemaphore
  instructions (Tile's own drain/barrier needs this too).
"""

import numpy as np
import ml_dtypes

import concourse.bass as bass
import concourse.mybir as mybir
import concourse.tile as tile
from concourse.bass_utils import run_bass_kernel_spmd
from concourse.masks import make_identity

D = 128          # num variables
H = 16           # hidden dim
B = 8192         # batch
N_CORES = 8
BSH = B // N_CORES       # 1024 batch rows per core
BT = 512                 # batch tile (free axis of matmuls)
NBT = BSH // BT          # 2 batch tiles per core
NCH = (D * H) // 128     # 16 chunks of 128 (i,h) pairs; 8 vars per chunk
VPC = 128 // H           # 8 variables per chunk
RPP = BSH // 128         # 8 batch rows per SBUF partition (interleave)

F32 = mybir.dt.float32
BF16 = mybir.dt.bfloat16


def _build_bass():
    nc = bass.Bass()

    # xt: X already transposed+interleaved on host: xt[v, f] with
    # f = bt*512 + s*128 + q  <->  batch row 8q + 4*bt + s   (2KB runs)
    xt_in = nc.declare_dram_parameter("xt", [D, BSH], BF16, isOutput=False)
    # A-chain inputs ship bf16 (like every other matmul input):
    # smallb cols 0:128 adj, 128:256 g0, 256:384 g1
    # smallf cols 0:16 b1c, 16 b2c, 17 temp   (fp32 biases)
    smallb = nc.declare_dram_parameter("smallb", [128, 384], BF16, isOutput=False)
    smallf = nc.declare_dram_parameter("smallf", [128, 32], F32, isOutput=False)
    w1t = nc.declare_dram_parameter("w1t", [D, D * H], BF16, isOutput=False)
    w2bd = nc.declare_dram_parameter("w2bd", [128, NCH * 32], BF16, isOutput=False)

    # recon is written TRANSPOSED [i, f]; the host un-transposes during
    # the unshard step (layout only, like the input interleave).
    recon_out = nc.declare_dram_parameter("recon_t", [D, BSH], F32, isOutput=True)
    a_out = nc.declare_dram_parameter("a_out", [D, D], F32, isOutput=True)

    with tile.TileContext(nc) as tc:
        with (
            tc.tile_pool(name="singles", bufs=1) as singles,
            tc.tile_pool(name="hid", bufs=6) as hid_pool,
            tc.tile_pool(name="outs", bufs=2) as out_pool,
            tc.tile_pool(name="pstb", bufs=3, space="PSUM") as pstb,
            tc.tile_pool(name="ps1", bufs=3, space="PSUM") as ps1_pool,
            tc.tile_pool(name="ps2", bufs=2, space="PSUM") as ps2_pool,
        ):
            # ---------------- setup DMAs (split across both HWDGE rings) ----
            smallb_sb = singles.tile([128, 384], BF16)
            smallf_sb = singles.tile([128, 32], F32)
            # (1 - eye) mask for the A output -- W1eff itself doesn't need it
            # because W1t ships with its diagonal (v == i) entries zeroed.
            eyec_sb = singles.tile([128, 128], F32)
            nc.gpsimd.memset(eyec_sb[:], 1.0)
            nc.gpsimd.affine_select(
                out=eyec_sb[:], in_=eyec_sb[:],
                compare_op=mybir.AluOpType.not_equal,
                fill=0.0, base=0, pattern=[[-1, D]], channel_multiplier=1)
            w1t_sb = singles.tile([D, D * H], BF16)
            w2bd_sb = singles.tile([128, NCH * 32], BF16)
            xt_sb = singles.tile([128, BSH], BF16)

            # sync ring: small (gates the A chain), then w1t in quarters
            # (W1eff chunk c only waits quarter c//4).  ACT ring: xt, w2bd.
            nc.sync.dma_start(smallb_sb[:], smallb[:])
            nc.sync.dma_start(smallf_sb[:], smallf[:])
            for q in range(4):
                qs = slice(q * 512, (q + 1) * 512)
                nc.sync.dma_start(w1t_sb[:, qs], w1t[:, qs])
            nc.scalar.dma_start(xt_sb[:], xt_in[:])
            nc.scalar.dma_start(w2bd_sb[:], w2bd[:])

            adj_sb = smallb_sb[:, 0:128]
            g0_sb = smallb_sb[:, 128:256]
            g1_sb = smallb_sb[:, 256:384]
            b1_sb = smallf_sb[:, 0:16]
            b2_sb = smallf_sb[:, 16:17]
            t_sb = smallf_sb[:, 17:18]

            # PE warmup: transposes of a zeroed scratch tile keep the PE busy
            # from right after the preamble, so the HAM clock gate is at
            # 2.4 GHz by the time real matmuls arrive (data-independent).
            warm_sb = singles.tile([128, 128], BF16)
            nc.gpsimd.memset(warm_sb[:], 0)
            for _w in range(45):
                ptw = pstb.tile([128, 128], BF16, tag="pt")
                nc.tensor.transpose(ptw[:], warm_sb[:], warm_sb[:])

            # ACT observes the `small` DMA early so later ACT ops reading
            # b1/b2 don't need a fresh DMA wait (1 wait slot per inst).
            act_scratch = singles.tile([128, 1], F32)
            nc.scalar.copy(act_scratch[:], t_sb)

            # ---------------- adjacency A (DVE + one ACT sigmoid) ----------
            z_sb = singles.tile([D, D], F32)
            nc.vector.tensor_sub(z_sb[:], g1_sb, g0_sb)
            nc.vector.scalar_tensor_tensor(
                out=z_sb[:], in0=adj_sb, scalar=2.0, in1=z_sb[:],
                op0=mybir.AluOpType.mult, op1=mybir.AluOpType.add)
            trec = singles.tile([128, 1], F32)
            nc.vector.reciprocal(trec[:], t_sb)
            a_sb = singles.tile([D, D], F32)
            nc.scalar.activation(
                a_sb[:], z_sb[:], mybir.ActivationFunctionType.Sigmoid,
                scale=trec[:, 0:1],
            )
            # masked copy of A only for the a_out result (off critical path)
            a_m = singles.tile([D, D], F32)
            nc.vector.tensor_mul(a_m[:], a_sb[:], eyec_sb[:])
            nc.sync.dma_start(a_out[:], a_m[:])

            # ---------------- W1eff = W1t * A (free-dim bcast over h) -------
            w1e_sb = singles.tile([D, D * H], BF16)
            for c in range(NCH):
                sl = slice(c * 128, (c + 1) * 128)
                in0 = w1t_sb[:, sl].rearrange("p (i h) -> p i h", h=H)
                out0 = w1e_sb[:, sl].rearrange("p (i h) -> p i h", h=H)
                off = 32 * (c % 4) + VPC * (c // 4)
                a_bc = a_sb[:, off:off + VPC, None].to_broadcast(
                    (D, VPC, H))
                eng = nc.vector if c < 4 else nc.gpsimd
                eng.tensor_mul(out0, in0, a_bc)

            # ---------------- main loop over batch tiles ----------------
            def mm2_round(ps2, hids, r):
                # 4 concurrent column-tiled matmuls: chunk 4r+g -> col group g
                for g in range(4):
                    c = 4 * r + g
                    nc.tensor.matmul(
                        ps2[32 * g:32 * (g + 1), :],
                        w2bd_sb[:, c * 32:(c + 1) * 32],
                        hids[c][:],
                        start=(r == 0), stop=(r == 3),
                        tile_position=(0, 32 * g),
                        skip_group_check=True)

            def epilogue(bt, ps2):
                # reconT[i, f] = psum2 + b2, stored transposed (fp32)
                reconT = out_pool.tile([128, BT], F32, tag="rT")
                nc.vector.tensor_scalar_add(reconT[:], ps2[:], b2_sb)
                eng = nc.sync if bt == 0 else nc.scalar
                eng.dma_start(recon_out[:, bt * BT:(bt + 1) * BT], reconT[:])

            prev_ps2 = None
            for bt in range(NBT):
                xt_bt = xt_sb[:, bt * BT:(bt + 1) * BT]
                ps2 = ps2_pool.tile([128, BT], F32)
                hids = []
                rounds_done = 0
                for c in range(NCH):
                    sl = slice(c * 128, (c + 1) * 128)
                    ps1 = ps1_pool.tile([128, BT], F32)
                    nc.tensor.matmul(
                        ps1[:], w1e_sb[:, sl], xt_bt,
                        start=True, stop=True)
                    hid = hid_pool.tile([128, BT], BF16)
                    if c % 2 == 1:
                        nc.vector.tensor_scalar(
                            hid[:], ps1[:],
                            scalar1=b1_sb[:, c:c + 1], scalar2=0.0,
                            op0=mybir.AluOpType.add, op1=mybir.AluOpType.max)
                    else:
                        nc.scalar.activation(
                            hid[:], ps1[:], mybir.ActivationFunctionType.Relu,
                            bias=b1_sb[:, c:c + 1])
                    hids.append(hid)
                    # previous tile's epilogue slots in behind our 3rd chunk
                    # so its transposes don't stall the PE at the boundary
                    if c == 2 and prev_ps2 is not None:
                        epilogue(bt - 1, prev_ps2)
                        prev_ps2 = None
                    # run mm2 round r once its 4 chunks have ~2 chunks slack
                    if c == 4 * rounds_done + 5:
                        mm2_round(ps2, hids, rounds_done)
                        rounds_done += 1
                while rounds_done < 4:
                    mm2_round(ps2, hids, rounds_done)
                    rounds_done += 1
                prev_ps2 = ps2
            epilogue(NBT - 1, prev_ps2)

    return nc


def _split_waits(nc):
    """This container's walrus encodes at most ONE sync-wait command per
    instruction; hoist extra waits into standalone EventSemaphore insts
    placed just before the owner (same engine, same block order)."""
    uid = 0
    for fn in nc.m.functions:
        for blk in fn.blocks:
            out = []
            for inst in blk.instructions:
                si = inst.sync_info
                if si is not None and si.on_wait is not None and len(si.on_wait) > 1:
                    waits = list(si.on_wait)
                    for w in waits[:-1]:
                        uid += 1
                        out.append(mybir.InstEventSemaphore(
                            name=f"I-waitsplit-{uid}",
                            engine=inst.engine,
                            ins=[], outs=[],
                            sync_info=mybir.SyncInfo(on_wait=[w], on_update=[]),
                        ))
                    inst.sync_info = mybir.SyncInfo(
                        on_wait=[waits[-1]], on_update=list(si.on_update or []))
                out.append(inst)
            blk.instructions = out
    return nc


_NC_CACHE = None


def _get_nc():
    global _NC_CACHE
    if _NC_CACHE is None:
        _NC_CACHE = _split_waits(_build_bass())
    return _NC_CACHE


def _prep_host_inputs(X, adjacency_logits, temperature, gumbel, W1, b1, W2, b2):
    """Layout transforms + bf16 casts shared by every core."""
    f32 = np.float32
    bf16 = ml_dtypes.bfloat16
    X = np.asarray(X, dtype=f32)
    adj = np.ascontiguousarray(np.asarray(adjacency_logits, dtype=f32))
    gum = np.asarray(gumbel, dtype=f32)
    g0 = np.ascontiguousarray(gum[:, :, 0])
    g1 = np.ascontiguousarray(gum[:, :, 1])
    t = np.asarray(temperature, dtype=f32).reshape(-1)[0]
    W1 = np.asarray(W1, dtype=f32)
    b1 = np.asarray(b1, dtype=f32)
    # chunk c owns vars off(c)..off(c)+7 with off(c)=32*(c%4)+8*(c//4), so
    # chunk c's mm2 output lands in PE column group c%4 (4-way col tiling).
    chunk_vars = np.array(
        [32 * (c % 4) + VPC * (c // 4) + il
         for c in range(NCH) for il in range(VPC)])  # [NCH*VPC]
    # W1t [v, c*128 + il*16 + h], with the (v == i) diagonal zeroed so the
    # device-side W1eff = W1t * A_raw needs no separate diagonal mask
    w1tp = W1.transpose(1, 0, 2)[:, chunk_vars, :].copy()  # [v, NCH*VPC, H]
    w1tp[np.arange(D)[:, None] == chunk_vars[None, :], :] = 0.0
    w1t = np.ascontiguousarray(w1tp.reshape(D, D * H).astype(bf16))
    # b1c [il*H+h, c]
    b1c = np.ascontiguousarray(
        b1[chunk_vars, :].reshape(NCH, VPC, H).transpose(1, 2, 0)
        .reshape(128, NCH))
    smallb = np.zeros((128, 384), dtype=bf16)
    smallb[:, 0:128] = adj.astype(bf16)
    smallb[:, 128:256] = g0.astype(bf16)
    smallb[:, 256:384] = g1.astype(bf16)
    smallf = np.zeros((128, 32), dtype=f32)
    smallf[:, 0:16] = b1c
    smallf[:, 16] = np.asarray(b2, dtype=f32)
    smallf[:, 17] = t
    W2 = np.asarray(W2, dtype=f32)
    # col-tiled stationary for mm2: chunk c -> [128, 32] block, column
    # (i - 32*(c%4)) = 8*(c//4)+il within col group c%4
    w2bd = np.zeros((128, NCH, 32), dtype=f32)
    for c in range(NCH):
        for il in range(VPC):
            i = 32 * (c % 4) + VPC * (c // 4) + il
            w2bd[il * H:(il + 1) * H, c, VPC * (c // 4) + il] = W2[i, :]
    w2bd = np.ascontiguousarray(w2bd.reshape(128, NCH * 32).astype(bf16))
    shared = dict(smallb=smallb, smallf=smallf, w1t=w1t, w2bd=w2bd)
    # xt per core: [v, f] with f = bt*512+s*128+q <-> row 8q+4bt+s
    Xb = (X.astype(bf16)
          .reshape(N_CORES, 128, NBT, BT // 128, D)   # [core, q, bt, s, v]
          .transpose(0, 4, 2, 3, 1)                   # [core, v, bt, s, q]
          .reshape(N_CORES, D, BSH))
    return np.ascontiguousarray(Xb), shared


def kernel(X, adjacency_logits, temperature, gumbel, W1, b1, W2, b2,
           _trace=False, _tmpdir=None):
    nc = _get_nc()
    Xb, shared = _prep_host_inputs(
        X, adjacency_logits, temperature, gumbel, W1, b1, W2, b2)
    in_maps = []
    for i in range(N_CORES):
        m = dict(shared)
        m["xt"] = np.ascontiguousarray(Xb[i])
        in_maps.append(m)
    res = run_bass_kernel_spmd(
        nc, in_maps, core_ids=list(range(N_CORES)),
        trace=_trace, tmpdir=_tmpdir,
    )
    # recon_t[i, bt*512+s*128+q] -> recon[q*8 + bt*4 + s, i] per core
    recon = np.concatenate([
        r["recon_t"].reshape(D, NBT, BT // 128, 128)
        .transpose(3, 1, 2, 0).reshape(BSH, D)
        for r in res.results], axis=0)
    A = res.results[0]["a_out"]
    kernel.last_exec_time_ns = res.exec_time_ns
    kernel.last_results = res
    return recon, A


# revision 39
# speedup vs baseline: 1.0099x; 1.0099x over previous
"""Trainium2 Bass kernel for the DifferentiableDAG forward pass.

reference math (D=128 vars, H=16 hidden, B=8192 batch):
    A[v,i]   = gumbel-sigmoid((2*logits[v,i] + g1 - g0)/t), zero diagonal
    hidden   = relu(einsum('biv,ivh', X[:,None,:]*A.T[None], W1) + b1)
    recon    = einsum('bih,ih', hidden, W2) + b2
    returns (recon, A)

Key identity: hidden_pre[b, i, h] = sum_v X[b,v] * (A[v,i]*W1[i,v,h]),
so the device builds W1eff[v, (i,h)] = A[v,i]*W1t[v, (i,h)] once and runs
a dense [B,128] @ [128,2048] matmul -- the [B,D,D] masked tensor from the
reference is never materialized (that's the memory-regime headroom).

Sharding: data-parallel on batch across 8 cores (1024 rows each);
adjacency/weights replicated, A and W1eff computed redundantly per core
(tiny). No collectives.

On-chip layout ("layout A"): (i_local*16+h) on PSUM partitions, batch on
the free axis, so b1/b2 are per-partition biases for the ACT engine.
  mm1:  psum1[128=(il,h), 512=b] = W1eff_chunk.T @ X.T    (16 chunks)
  relu: hid_c = relu(psum1 + b1_chunk)                    (ACT/DVE split)
  mm2:  psum2[128=i, 512=b] += W2blockdiag_chunk.T @ hid  (col-tiled 4x)
  out:  reconT = psum2 + b2, stored TRANSPOSED; host un-transposes.

Perf notes (measured on this container's trn2 + walrus):
- all matmul operands bf16 (fp32 runs two-pass at half rate); biases and
  PSUM accumulation stay fp32; recon error ~4e-3 scale-relative.
- X ships pre-transposed/interleaved and recon is written transposed
  (host un-transposes at unshard): no on-device transposes at all, and
  every DMA moves >=2KB contiguous runs per partition (small row-sized
  runs crawl at <5GB/s per queue from descriptor overhead).
- W1t ships with its (v==i) diagonal pre-zeroed, which takes the A-mask
  off the critical path; the masked A is produced separately for a_out.
- w1t streams in 4 quarters so mm1 chunk c only waits on quarter c//4;
  the A-chain inputs land first, packed and cast to bf16 (halves the
  gating DMA; A output error ~3e-3, still far under the gate).
- 40 dummy transposes of a zeroed scratch tile keep the PE busy across
  the whole DMA-bound ramp, so the HAM clock gate is at 2.4 GHz (and
  stays there) when the real matmul stream starts.
- mm2 uses tile_position column groups: chunk c owns vars
  off(c)=32*(c%4)+8*(c//4)..+8, so its 8 outputs land in col group c%4
  with a [128,32] stationary -- cheap loads, one accumulating psum.
- _split_waits(): this walrus encodes at most ONE sync-wait command per
  instruction, so extra waits are hoisted into standalone EventSemaphore
  instructions (Tile's own drain/barrier needs this too).
"""

import numpy as np
import ml_dtypes

import concourse.bass as bass
import concourse.mybir as mybir
import concourse.tile as tile
from concourse.bass_utils import run_bass_kernel_spmd
from concourse.masks import make_identity

D = 128          # num variables
H = 16           # hidden dim
B = 8192         # batch
N_CORES = 8
BSH = B // N_CORES       # 1024 batch rows per core
BT = 512                 # batch tile (free axis of matmuls)
NBT = BSH // BT          # 2 batch tiles per core
NCH = (D * H) // 128     # 16 chunks of 128 (i,h) pairs; 8 vars per chunk
VPC = 128 // H           # 8 variables per chunk
RPP = BSH // 128         # 8 batch rows per SBUF partition (interleave)

F32 = mybir.dt.float32
BF16 = mybir.dt.bfloat16


def _build_bass():
    nc = bass.Bass()

    # xt: X already transposed+interleaved on host: xt[v, f] with
    # f = bt*512 + s*128 + q  <->  batch row 8q + 4*bt + s   (2KB runs)
    xt_in = nc.declare_dram_parameter("xt", [D, BSH], BF16, isOutput=False)
    # A-chain inputs ship bf16 (like every other matmul input):
    # smallb cols 0:128 adj, 128:256 g0, 256:384 g1
    # smallf cols 0:16 b1c, 16 b2c, 17 temp   (fp32 biases)
    smallb = nc.declare_dram_parameter("smallb", [128, 384], BF16, isOutput=False)
    smallf = nc.declare_dram_parameter("smallf", [128, 32], F32, isOutput=False)
    w1t = nc.declare_dram_parameter("w1t", [D, D * H], BF16, isOutput=False)
    w2bd = nc.declare_dram_parameter("w2bd", [128, NCH * 32], BF16, isOutput=False)

    # recon is written TRANSPOSED [i, f]; the host un-transposes during
    # the unshard step (layout only, like the input interleave).
    recon_out = nc.declare_dram_parameter("recon_t", [D, BSH], F32, isOutput=True)
    a_out = nc.declare_dram_parameter("a_out", [D, D], F32, isOutput=True)

    with tile.TileContext(nc) as tc:
        with (
            tc.tile_pool(name="singles", bufs=1) as singles,
            tc.tile_pool(name="hid", bufs=6) as hid_pool,
            tc.tile_pool(name="outs", bufs=2) as out_pool,
            tc.tile_pool(name="pstb", bufs=3, space="PSUM") as pstb,
            tc.tile_pool(name="ps1", bufs=3, space="PSUM") as ps1_pool,
            tc.tile_pool(name="ps2", bufs=2, space="PSUM") as ps2_pool,
        ):
            # ---------------- setup DMAs (split across both HWDGE rings) ----
            smallb_sb = singles.tile([128, 384], BF16)
            smallf_sb = singles.tile([128, 32], F32)
            # (1 - eye) mask for the A output -- W1eff itself doesn't need it
            # because W1t ships with its diagonal (v == i) entries zeroed.
            eyec_sb = singles.tile([128, 128], F32)
            nc.gpsimd.memset(eyec_sb[:], 1.0)
            nc.gpsimd.affine_select(
                out=eyec_sb[:], in_=eyec_sb[:],
                compare_op=mybir.AluOpType.not_equal,
                fill=0.0, base=0, pattern=[[-1, D]], channel_multiplier=1)
            w1t_sb = singles.tile([D, D * H], BF16)
            w2bd_sb = singles.tile([128, NCH * 32], BF16)
            xt_sb = singles.tile([128, BSH], BF16)

            # sync ring: small (gates the A chain), then w1t in quarters
            # (W1eff chunk c only waits quarter c//4).  ACT ring: xt, w2bd.
            nc.sync.dma_start(smallb_sb[:], smallb[:])
            nc.sync.dma_start(smallf_sb[:], smallf[:])
            for q in range(4):
                qs = slice(q * 512, (q + 1) * 512)
                nc.sync.dma_start(w1t_sb[:, qs], w1t[:, qs])
            nc.scalar.dma_start(xt_sb[:], xt_in[:])
            nc.scalar.dma_start(w2bd_sb[:], w2bd[:])

            adj_sb = smallb_sb[:, 0:128]
            g0_sb = smallb_sb[:, 128:256]
            g1_sb = smallb_sb[:, 256:384]
            b1_sb = smallf_sb[:, 0:16]
            b2_sb = smallf_sb[:, 16:17]
            t_sb = smallf_sb[:, 17:18]

            # PE warmup: transposes of a zeroed scratch tile keep the PE busy
            # from right after the preamble, so the HAM clock gate is at
            # 2.4 GHz by the time real matmuls arrive (data-independent).
            warm_sb = singles.tile([128, 128], BF16)
            nc.gpsimd.memset(warm_sb[:], 0)
            for _w in range(40):
                ptw = pstb.tile([128, 128], BF16, tag="pt")
                nc.tensor.transpose(ptw[:], warm_sb[:], warm_sb[:])

            # ACT observes the `small` DMA early so later ACT ops reading
            # b1/b2 don't need a fresh DMA wait (1 wait slot per inst).
            act_scratch = singles.tile([128, 1], F32)
            nc.scalar.copy(act_scratch[:], t_sb)

            # ---------------- adjacency A (DVE + one ACT sigmoid) ----------
            z_sb = singles.tile([D, D], F32)
            nc.vector.tensor_sub(z_sb[:], g1_sb, g0_sb)
            nc.vector.scalar_tensor_tensor(
                out=z_sb[:], in0=adj_sb, scalar=2.0, in1=z_sb[:],
                op0=mybir.AluOpType.mult, op1=mybir.AluOpType.add)
            trec = singles.tile([128, 1], F32)
            nc.vector.reciprocal(trec[:], t_sb)
            a_sb = singles.tile([D, D], F32)
            nc.scalar.activation(
                a_sb[:], z_sb[:], mybir.ActivationFunctionType.Sigmoid,
                scale=trec[:, 0:1],
            )
            # masked copy of A only for the a_out result (off critical path)
            a_m = singles.tile([D, D], F32)
            nc.vector.tensor_mul(a_m[:], a_sb[:], eyec_sb[:])
            nc.sync.dma_start(a_out[:], a_m[:])

            # ---------------- W1eff = W1t * A (free-dim bcast over h) -------
            w1e_sb = singles.tile([D, D * H], BF16)
            for c in range(NCH):
                sl = slice(c * 128, (c + 1) * 128)
                in0 = w1t_sb[:, sl].rearrange("p (i h) -> p i h", h=H)
                out0 = w1e_sb[:, sl].rearrange("p (i h) -> p i h", h=H)
                off = 32 * (c % 4) + VPC * (c // 4)
                a_bc = a_sb[:, off:off + VPC, None].to_broadcast(
                    (D, VPC, H))
                eng = nc.vector if c < 4 else nc.gpsimd
                eng.tensor_mul(out0, in0, a_bc)

            # ---------------- main loop over batch tiles ----------------
            def mm2_round(ps2, hids, r):
                # 4 concurrent column-tiled matmuls: chunk 4r+g -> col group g
                for g in range(4):
                    c = 4 * r + g
                    nc.tensor.matmul(
                        ps2[32 * g:32 * (g + 1), :],
                        w2bd_sb[:, c * 32:(c + 1) * 32],
                        hids[c][:],
                        start=(r == 0), stop=(r == 3),
                        tile_position=(0, 32 * g),
                        skip_group_check=True)

            def epilogue(bt, ps2):
                # reconT[i, f] = psum2 + b2, stored transposed (fp32)
                reconT = out_pool.tile([128, BT], F32, tag="rT")
                nc.vector.tensor_scalar_add(reconT[:], ps2[:], b2_sb)
                eng = nc.sync if bt == 0 else nc.scalar
                eng.dma_start(recon_out[:, bt * BT:(bt + 1) * BT], reconT[:])

            prev_ps2 = None
            for bt in range(NBT):
                xt_bt = xt_sb[:, bt * BT:(bt + 1) * BT]
                ps2 = ps2_pool.tile([128, BT], F32)
                hids = []
                rounds_done = 0
                for c in range(NCH):
                    sl = slice(c * 128, (c + 1) * 128)
                    ps1 = ps1_pool.tile([128, BT], F32)
                    nc.tensor.matmul(
                        ps1[:], w1e_sb[:, sl], xt_bt,
                        start=True, stop=True)
                    hid = hid_pool.tile([128, BT], BF16)
                    if c % 2 == 1:
                        nc.vector.tensor_scalar(
                            hid[:], ps1[:],
                            scalar1=b1_sb[:, c:c + 1], scalar2=0.0,
                            op0=mybir.AluOpType.add, op1=mybir.AluOpType.max)
                    else:
                        nc.scalar.activation(
                            hid[:], ps1[:], mybir.ActivationFunctionType.Relu,
                            bias=b1_sb[:, c:c + 1])
                    hids.append(hid)
                    # previous tile's epilogue slots in behind our 3rd chunk
                    # so its transposes don't stall the PE at the boundary
                    if c == 2 and prev_ps2 is not None:
                        epilogue(bt - 1, prev_ps2)
                        prev_ps2 = None
                    # run mm2 round r once its 4 chunks have ~2 chunks slack
                    if c == 4 * rounds_done + 5:
                        mm2_round(ps2, hids, rounds_done)
                        rounds_done += 1
                while rounds_done < 4:
                    mm2_round(ps2, hids, rounds_done)
                    rounds_done += 1
                prev_ps2 = ps2
            epilogue(NBT - 1, prev_ps2)

    return nc


def _split_waits(nc):
    """This container's walrus encodes at most ONE sync-wait command per
    instruction; hoist extra waits into standalone EventSemaphore insts
    placed just before the owner (same engine, same block order)."""
    uid = 0
    for fn in nc.m.functions:
        for blk in fn.blocks:
            out = []
            for inst in blk.instructions:
                si = inst.sync_info
                if si is not None and si.on_wait is not None and len(si.on_wait) > 1:
                    waits = list(si.on_wait)
                    for w in waits[:-1]:
                        uid += 1
                        out.append(mybir.InstEventSemaphore(
                            name=f"I-waitsplit-{uid}",
                            engine=inst.engine,
                            ins=[], outs=[],
                            sync_info=mybir.SyncInfo(on_wait=[w], on_update=[]),
                        ))
                    inst.sync_info = mybir.SyncInfo(
                        on_wait=[waits[-1]], on_update=list(si.on_update or []))
                out.append(inst)
            blk.instructions = out
    return nc


_NC_CACHE = None


def _get_nc():
    global _NC_CACHE
    if _NC_CACHE is None:
        _NC_CACHE = _split_waits(_build_bass())
    return _NC_CACHE


def _prep_host_inputs(X, adjacency_logits, temperature, gumbel, W1, b1, W2, b2):
    """Layout transforms + bf16 casts shared by every core."""
    f32 = np.float32
    bf16 = ml_dtypes.bfloat16
    X = np.asarray(X, dtype=f32)
    adj = np.ascontiguousarray(np.asarray(adjacency_logits, dtype=f32))
    gum = np.asarray(gumbel, dtype=f32)
    g0 = np.ascontiguousarray(gum[:, :, 0])
    g1 = np.ascontiguousarray(gum[:, :, 1])
    t = np.asarray(temperature, dtype=f32).reshape(-1)[0]
    W1 = np.asarray(W1, dtype=f32)
    b1 = np.asarray(b1, dtype=f32)
    # chunk c owns vars off(c)..off(c)+7 with off(c)=32*(c%4)+8*(c//4), so
    # chunk c's mm2 output lands in PE column group c%4 (4-way col tiling).
    chunk_vars = np.array(
        [32 * (c % 4) + VPC * (c // 4) + il
         for c in range(NCH) for il in range(VPC)])  # [NCH*VPC]
    # W1t [v, c*128 + il*16 + h], with the (v == i) diagonal zeroed so the
    # device-side W1eff = W1t * A_raw needs no separate diagonal mask
    w1tp = W1.transpose(1, 0, 2)[:, chunk_vars, :].copy()  # [v, NCH*VPC, H]
    w1tp[np.arange(D)[:, None] == chunk_vars[None, :], :] = 0.0
    w1t = np.ascontiguousarray(w1tp.reshape(D, D * H).astype(bf16))
    # b1c [il*H+h, c]
    b1c = np.ascontiguousarray(
        b1[chunk_vars, :].reshape(NCH, VPC, H).transpose(1, 2, 0)
        .reshape(128, NCH))
    smallb = np.zeros((128, 384), dtype=bf16)
    smallb[:, 0:128] = adj.astype(bf16)
    smallb[:, 128:256] = g0.astype(bf16)
    smallb[:, 256:384] = g1.astype(bf16)
    smallf = np.zeros((128, 32), dtype=f32)
    smallf[:, 0:16] = b1c
    smallf[:, 16] = np.asarray(b2, dtype=f32)
    smallf[:, 17] = t
    W2 = np.asarray(W2, dtype=f32)
    # col-tiled stationary for mm2: chunk c -> [128, 32] block, column
    # (i - 32*(c%4)) = 8*(c//4)+il within col group c%4
    w2bd = np.zeros((128, NCH, 32), dtype=f32)
    for c in range(NCH):
        for il in range(VPC):
            i = 32 * (c % 4) + VPC * (c // 4) + il
            w2bd[il * H:(il + 1) * H, c, VPC * (c // 4) + il] = W2[i, :]
    w2bd = np.ascontiguousarray(w2bd.reshape(128, NCH * 32).astype(bf16))
    shared = dict(smallb=smallb, smallf=smallf, w1t=w1t, w2bd=w2bd)
    # xt per core: [v, f] with f = bt*512+s*128+q <-> row 8q+4bt+s
    Xb = (X.astype(bf16)
          .reshape(N_CORES, 128, NBT, BT // 128, D)   # [core, q, bt, s, v]
          .transpose(0, 4, 2, 3, 1)                   # [core, v, bt, s, q]
          .reshape(N_CORES, D, BSH))
    return np.ascontiguousarray(Xb), shared


def kernel(X, adjacency_logits, temperature, gumbel, W1, b1, W2, b2,
           _trace=False, _tmpdir=None):
    nc = _get_nc()
    Xb, shared = _prep_host_inputs(
        X, adjacency_logits, temperature, gumbel, W1, b1, W2, b2)
    in_maps = []
    for i in range(N_CORES):
        m = dict(shared)
        m["xt"] = np.ascontiguousarray(Xb[i])
        in_maps.append(m)
    res = run_bass_kernel_spmd(
        nc, in_maps, core_ids=list(range(N_CORES)),
        trace=_trace, tmpdir=_tmpdir,
    )
    # recon_t[i, bt*512+s*128+q] -> recon[q*8 + bt*4 + s, i] per core
    recon = np.concatenate([
        r["recon_t"].reshape(D, NBT, BT // 128, 128)
        .transpose(3, 1, 2, 0).reshape(BSH, D)
        for r in res.results], axis=0)
    A = res.results[0]["a_out"]
    kernel.last_exec_time_ns = res.exec_time_ns
    kernel.last_results = res
    return recon, A


# revision 40
# speedup vs baseline: 1.0979x; 1.0871x over previous
"""Trainium2 Bass kernel for the DifferentiableDAG forward pass.

reference math (D=128 vars, H=16 hidden, B=8192 batch):
    A[v,i]   = gumbel-sigmoid((2*logits[v,i] + g1 - g0)/t), zero diagonal
    hidden   = relu(einsum('biv,ivh', X[:,None,:]*A.T[None], W1) + b1)
    recon    = einsum('bih,ih', hidden, W2) + b2
    returns (recon, A)

Key identity: hidden_pre[b, i, h] = sum_v X[b,v] * (A[v,i]*W1[i,v,h]),
so the device builds W1eff[v, (i,h)] = A[v,i]*W1t[v, (i,h)] once and runs
a dense [B,128] @ [128,2048] matmul -- the [B,D,D] masked tensor from the
reference is never materialized (that's the memory-regime headroom).

Sharding: data-parallel on batch across 8 cores (1024 rows each);
adjacency/weights replicated, A and W1eff computed redundantly per core
(tiny). No collectives.

On-chip layout ("layout A"): (i_local*16+h) on PSUM partitions, batch on
the free axis, so b1/b2 are per-partition biases for the ACT engine.
  mm1:  psum1[128=(il,h), 512=b] = W1eff_chunk.T @ X.T    (16 chunks)
  relu: hid_c = relu(psum1 + b1_chunk)                    (ACT/DVE split)
  mm2:  psum2[128=i, 512=b] += W2blockdiag_chunk.T @ hid  (col-tiled 4x)
  out:  reconT = psum2 + b2, stored TRANSPOSED; host un-transposes.

Perf notes (measured on this container's trn2 + walrus):
- all matmul operands bf16 (fp32 runs two-pass at half rate); biases and
  PSUM accumulation stay fp32; recon error ~4e-3 scale-relative.
- X ships pre-transposed/interleaved and recon is written transposed
  (host un-transposes at unshard): no on-device transposes at all, and
  every DMA moves >=2KB contiguous runs per partition (small row-sized
  runs crawl at <5GB/s per queue from descriptor overhead).
- W1t ships with its (v==i) diagonal pre-zeroed, which takes the A-mask
  off the critical path; the masked A is produced separately for a_out.
- w1t streams in 4 quarters so mm1 chunk c only waits on quarter c//4;
  the A-chain inputs land first, packed and cast to bf16 (halves the
  gating DMA; A output error ~3e-3, still far under the gate).
- 40 dummy transposes of a zeroed scratch tile keep the PE busy across
  the whole DMA-bound ramp, so the HAM clock gate is at 2.4 GHz (and
  stays there) when the real matmul stream starts.
- mm2 uses tile_position column groups: chunk c owns vars
  off(c)=32*(c%4)+8*(c//4)..+8, so its 8 outputs land in col group c%4
  with a [128,32] stationary -- cheap loads, one accumulating psum.
- _split_waits(): this walrus encodes at most ONE sync-wait command per
  instruction, so extra waits are hoisted into standalone EventSemaphore
  instructions (Tile's own drain/barrier needs this too).
"""

import numpy as np
import ml_dtypes

import concourse.bass as bass
import concourse.mybir as mybir
import concourse.tile as tile
from concourse.bass_utils import run_bass_kernel_spmd
from concourse.masks import make_identity

D = 128          # num variables
H = 16           # hidden dim
B = 8192         # batch
N_CORES = 8
BSH = B // N_CORES       # 1024 batch rows per core
BT = 512                 # batch tile (free axis of matmuls)
NBT = BSH // BT          # 2 batch tiles per core
NCH = (D * H) // 128     # 16 chunks of 128 (i,h) pairs; 8 vars per chunk
VPC = 128 // H           # 8 variables per chunk
RPP = BSH // 128         # 8 batch rows per SBUF partition (interleave)

F32 = mybir.dt.float32
BF16 = mybir.dt.bfloat16


def _build_bass():
    nc = bass.Bass()

    # xt: X already transposed+interleaved on host: xt[v, f] with
    # f = bt*512 + s*128 + q  <->  batch row 8q + 4*bt + s   (2KB runs)
    xt_in = nc.declare_dram_parameter("xt", [D, BSH], BF16, isOutput=False)
    # A-chain inputs ship bf16 (like every other matmul input):
    # smallb cols 0:128 adj, 128:256 g0, 256:384 g1
    # smallf cols 0:16 b1c, 16 b2c, 17 temp   (fp32 biases)
    smallb = nc.declare_dram_parameter("smallb", [128, 384], BF16, isOutput=False)
    smallf = nc.declare_dram_parameter("smallf", [128, 32], F32, isOutput=False)
    w1t = nc.declare_dram_parameter("w1t", [D, D * H], BF16, isOutput=False)
    w2bd = nc.declare_dram_parameter("w2bd", [128, NCH * 32], BF16, isOutput=False)

    # recon is written TRANSPOSED [i, f]; the host un-transposes during
    # the unshard step (layout only, like the input interleave).
    recon_out = nc.declare_dram_parameter("recon_t", [D, BSH], BF16, isOutput=True)
    a_out = nc.declare_dram_parameter("a_out", [D, D], F32, isOutput=True)

    with tile.TileContext(nc) as tc:
        with (
            tc.tile_pool(name="singles", bufs=1) as singles,
            tc.tile_pool(name="hid", bufs=6) as hid_pool,
            tc.tile_pool(name="outs", bufs=2) as out_pool,
            tc.tile_pool(name="pstb", bufs=3, space="PSUM") as pstb,
            tc.tile_pool(name="ps1", bufs=3, space="PSUM") as ps1_pool,
            tc.tile_pool(name="ps2", bufs=2, space="PSUM") as ps2_pool,
        ):
            # ---------------- setup DMAs (split across both HWDGE rings) ----
            smallb_sb = singles.tile([128, 384], BF16)
            smallf_sb = singles.tile([128, 32], F32)
            # (1 - eye) mask for the A output -- W1eff itself doesn't need it
            # because W1t ships with its diagonal (v == i) entries zeroed.
            eyec_sb = singles.tile([128, 128], F32)
            nc.gpsimd.memset(eyec_sb[:], 1.0)
            nc.gpsimd.affine_select(
                out=eyec_sb[:], in_=eyec_sb[:],
                compare_op=mybir.AluOpType.not_equal,
                fill=0.0, base=0, pattern=[[-1, D]], channel_multiplier=1)
            w1t_sb = singles.tile([D, D * H], BF16)
            w2bd_sb = singles.tile([128, NCH * 32], BF16)
            xt_sb = singles.tile([128, BSH], BF16)

            # sync ring: small (gates the A chain), then w1t in quarters
            # (W1eff chunk c only waits quarter c//4).  ACT ring: xt, w2bd.
            nc.sync.dma_start(smallb_sb[:], smallb[:])
            nc.sync.dma_start(smallf_sb[:], smallf[:])
            for q in range(4):
                qs = slice(q * 512, (q + 1) * 512)
                nc.sync.dma_start(w1t_sb[:, qs], w1t[:, qs])
            nc.scalar.dma_start(xt_sb[:], xt_in[:])
            nc.scalar.dma_start(w2bd_sb[:], w2bd[:])

            adj_sb = smallb_sb[:, 0:128]
            g0_sb = smallb_sb[:, 128:256]
            g1_sb = smallb_sb[:, 256:384]
            b1_sb = smallf_sb[:, 0:16]
            b2_sb = smallf_sb[:, 16:17]
            t_sb = smallf_sb[:, 17:18]

            # PE warmup: transposes of a zeroed scratch tile keep the PE busy
            # from right after the preamble, so the HAM clock gate is at
            # 2.4 GHz by the time real matmuls arrive (data-independent).
            warm_sb = singles.tile([128, 128], BF16)
            nc.gpsimd.memset(warm_sb[:], 0)
            for _w in range(40):
                ptw = pstb.tile([128, 128], BF16, tag="pt")
                nc.tensor.transpose(ptw[:], warm_sb[:], warm_sb[:])

            # ACT observes the `small` DMA early so later ACT ops reading
            # b1/b2 don't need a fresh DMA wait (1 wait slot per inst).
            act_scratch = singles.tile([128, 1], F32)
            nc.scalar.copy(act_scratch[:], t_sb)

            # ---------------- adjacency A (DVE + one ACT sigmoid) ----------
            z_sb = singles.tile([D, D], F32)
            nc.vector.tensor_sub(z_sb[:], g1_sb, g0_sb)
            nc.vector.scalar_tensor_tensor(
                out=z_sb[:], in0=adj_sb, scalar=2.0, in1=z_sb[:],
                op0=mybir.AluOpType.mult, op1=mybir.AluOpType.add)
            trec = singles.tile([128, 1], F32)
            nc.vector.reciprocal(trec[:], t_sb)
            a_sb = singles.tile([D, D], F32)
            nc.scalar.activation(
                a_sb[:], z_sb[:], mybir.ActivationFunctionType.Sigmoid,
                scale=trec[:, 0:1],
            )
            # masked copy of A only for the a_out result (off critical path)
            a_m = singles.tile([D, D], F32)
            nc.vector.tensor_mul(a_m[:], a_sb[:], eyec_sb[:])
            nc.sync.dma_start(a_out[:], a_m[:])

            # ---------------- W1eff = W1t * A (free-dim bcast over h) -------
            w1e_sb = singles.tile([D, D * H], BF16)
            for c in range(NCH):
                sl = slice(c * 128, (c + 1) * 128)
                in0 = w1t_sb[:, sl].rearrange("p (i h) -> p i h", h=H)
                out0 = w1e_sb[:, sl].rearrange("p (i h) -> p i h", h=H)
                off = 32 * (c % 4) + VPC * (c // 4)
                a_bc = a_sb[:, off:off + VPC, None].to_broadcast(
                    (D, VPC, H))
                eng = nc.vector if c < 4 else nc.gpsimd
                eng.tensor_mul(out0, in0, a_bc)

            # ---------------- main loop over batch tiles ----------------
            def mm2_round(ps2, hids, r):
                # 4 concurrent column-tiled matmuls: chunk 4r+g -> col group g
                for g in range(4):
                    c = 4 * r + g
                    nc.tensor.matmul(
                        ps2[32 * g:32 * (g + 1), :],
                        w2bd_sb[:, c * 32:(c + 1) * 32],
                        hids[c][:],
                        start=(r == 0), stop=(r == 3),
                        tile_position=(0, 32 * g),
                        skip_group_check=True)

            def epilogue(bt, ps2):
                # reconT[i, f] = psum2 + b2, stored transposed (fp32)
                reconT = out_pool.tile([128, BT], BF16, tag="rT")
                nc.vector.tensor_scalar_add(reconT[:], ps2[:], b2_sb)
                eng = nc.sync if bt == 0 else nc.scalar
                eng.dma_start(recon_out[:, bt * BT:(bt + 1) * BT], reconT[:])

            prev_ps2 = None
            for bt in range(NBT):
                xt_bt = xt_sb[:, bt * BT:(bt + 1) * BT]
                ps2 = ps2_pool.tile([128, BT], F32)
                hids = []
                rounds_done = 0
                for c in range(NCH):
                    sl = slice(c * 128, (c + 1) * 128)
                    ps1 = ps1_pool.tile([128, BT], F32)
                    nc.tensor.matmul(
                        ps1[:], w1e_sb[:, sl], xt_bt,
                        start=True, stop=True)
                    hid = hid_pool.tile([128, BT], BF16)
                    if c % 2 == 1:
                        nc.vector.tensor_scalar(
                            hid[:], ps1[:],
                            scalar1=b1_sb[:, c:c + 1], scalar2=0.0,
                            op0=mybir.AluOpType.add, op1=mybir.AluOpType.max)
                    else:
                        nc.scalar.activation(
                            hid[:], ps1[:], mybir.ActivationFunctionType.Relu,
                            bias=b1_sb[:, c:c + 1])
                    hids.append(hid)
                    # previous tile's epilogue slots in behind our 3rd chunk
                    # so its transposes don't stall the PE at the boundary
                    if c == 2 and prev_ps2 is not None:
                        epilogue(bt - 1, prev_ps2)
                        prev_ps2 = None
                    # run mm2 round r once its 4 chunks have ~2 chunks slack
                    if c == 4 * rounds_done + 5:
                        mm2_round(ps2, hids, rounds_done)
                        rounds_done += 1
                while rounds_done < 4:
                    mm2_round(ps2, hids, rounds_done)
                    rounds_done += 1
                prev_ps2 = ps2
            epilogue(NBT - 1, prev_ps2)

    return nc


def _split_waits(nc):
    """This container's walrus encodes at most ONE sync-wait command per
    instruction; hoist extra waits into standalone EventSemaphore insts
    placed just before the owner (same engine, same block order)."""
    uid = 0
    for fn in nc.m.functions:
        for blk in fn.blocks:
            out = []
            for inst in blk.instructions:
                si = inst.sync_info
                if si is not None and si.on_wait is not None and len(si.on_wait) > 1:
                    waits = list(si.on_wait)
                    for w in waits[:-1]:
                        uid += 1
                        out.append(mybir.InstEventSemaphore(
                            name=f"I-waitsplit-{uid}",
                            engine=inst.engine,
                            ins=[], outs=[],
                            sync_info=mybir.SyncInfo(on_wait=[w], on_update=[]),
                        ))
                    inst.sync_info = mybir.SyncInfo(
                        on_wait=[waits[-1]], on_update=list(si.on_update or []))
                out.append(inst)
            blk.instructions = out
    return nc


_NC_CACHE = None


def _get_nc():
    global _NC_CACHE
    if _NC_CACHE is None:
        _NC_CACHE = _split_waits(_build_bass())
    return _NC_CACHE


def _prep_host_inputs(X, adjacency_logits, temperature, gumbel, W1, b1, W2, b2):
    """Layout transforms + bf16 casts shared by every core."""
    f32 = np.float32
    bf16 = ml_dtypes.bfloat16
    X = np.asarray(X, dtype=f32)
    adj = np.ascontiguousarray(np.asarray(adjacency_logits, dtype=f32))
    gum = np.asarray(gumbel, dtype=f32)
    g0 = np.ascontiguousarray(gum[:, :, 0])
    g1 = np.ascontiguousarray(gum[:, :, 1])
    t = np.asarray(temperature, dtype=f32).reshape(-1)[0]
    W1 = np.asarray(W1, dtype=f32)
    b1 = np.asarray(b1, dtype=f32)
    # chunk c owns vars off(c)..off(c)+7 with off(c)=32*(c%4)+8*(c//4), so
    # chunk c's mm2 output lands in PE column group c%4 (4-way col tiling).
    chunk_vars = np.array(
        [32 * (c % 4) + VPC * (c // 4) + il
         for c in range(NCH) for il in range(VPC)])  # [NCH*VPC]
    # W1t [v, c*128 + il*16 + h], with the (v == i) diagonal zeroed so the
    # device-side W1eff = W1t * A_raw needs no separate diagonal mask
    w1tp = W1.transpose(1, 0, 2)[:, chunk_vars, :].copy()  # [v, NCH*VPC, H]
    w1tp[np.arange(D)[:, None] == chunk_vars[None, :], :] = 0.0
    w1t = np.ascontiguousarray(w1tp.reshape(D, D * H).astype(bf16))
    # b1c [il*H+h, c]
    b1c = np.ascontiguousarray(
        b1[chunk_vars, :].reshape(NCH, VPC, H).transpose(1, 2, 0)
        .reshape(128, NCH))
    smallb = np.zeros((128, 384), dtype=bf16)
    smallb[:, 0:128] = adj.astype(bf16)
    smallb[:, 128:256] = g0.astype(bf16)
    smallb[:, 256:384] = g1.astype(bf16)
    smallf = np.zeros((128, 32), dtype=f32)
    smallf[:, 0:16] = b1c
    smallf[:, 16] = np.asarray(b2, dtype=f32)
    smallf[:, 17] = t
    W2 = np.asarray(W2, dtype=f32)
    # col-tiled stationary for mm2: chunk c -> [128, 32] block, column
    # (i - 32*(c%4)) = 8*(c//4)+il within col group c%4
    w2bd = np.zeros((128, NCH, 32), dtype=f32)
    for c in range(NCH):
        for il in range(VPC):
            i = 32 * (c % 4) + VPC * (c // 4) + il
            w2bd[il * H:(il + 1) * H, c, VPC * (c // 4) + il] = W2[i, :]
    w2bd = np.ascontiguousarray(w2bd.reshape(128, NCH * 32).astype(bf16))
    shared = dict(smallb=smallb, smallf=smallf, w1t=w1t, w2bd=w2bd)
    # xt per core: [v, f] with f = bt*512+s*128+q <-> row 8q+4bt+s
    Xb = (X.astype(bf16)
          .reshape(N_CORES, 128, NBT, BT // 128, D)   # [core, q, bt, s, v]
          .transpose(0, 4, 2, 3, 1)                   # [core, v, bt, s, q]
          .reshape(N_CORES, D, BSH))
    return np.ascontiguousarray(Xb), shared


def kernel(X, adjacency_logits, temperature, gumbel, W1, b1, W2, b2,
           _trace=False, _tmpdir=None):
    nc = _get_nc()
    Xb, shared = _prep_host_inputs(
        X, adjacency_logits, temperature, gumbel, W1, b1, W2, b2)
    in_maps = []
    for i in range(N_CORES):
        m = dict(shared)
        m["xt"] = np.ascontiguousarray(Xb[i])
        in_maps.append(m)
    res = run_bass_kernel_spmd(
        nc, in_maps, core_ids=list(range(N_CORES)),
        trace=_trace, tmpdir=_tmpdir,
    )
    # recon_t[i, bt*512+s*128+q] -> recon[q*8 + bt*4 + s, i] per core
    recon = np.concatenate([
        r["recon_t"].astype(np.float32).reshape(D, NBT, BT // 128, 128)
        .transpose(3, 1, 2, 0).reshape(BSH, D)
        for r in res.results], axis=0)
    A = res.results[0]["a_out"]
    kernel.last_exec_time_ns = res.exec_time_ns
    kernel.last_results = res
    return recon, A
